# revision 2
# baseline (speedup 1.0000x reference)
"""Concordance CC (segment_reduce) Trainium2 Bass kernel — V7.

Problem: y_true, y_pred [256, 65536] f32, 0/1 validity mask [256, 65536] i32.
Per row: masked means/variances/covariance (ddof=1), ccc = 2*cov /
(var_t + var_p + 2*(mean_t - mean_p)); output = mean(ccc) (scalar f32).

Strategy (data parallel over B, 8 cores x 32 rows):
Per-row stats are inner products over T of columns from
W = [a_0..15, b_0..15, ones, a_16..31, b_16..31] with a = y_true*mask,
b = y_pred*mask:
  S2t=a.a  Stp=a.b  S1t=a.ones  S2p=b.b  S1p=b.ones
One 65x65 Gram W^T W per core on the TensorEngine, PSUM-accumulated over
all 512 T-chunks. All 32 rows share one Gram because the PE cost is
per-chunk LdWeights (128 rows @ 2/cycle, ~31 ns) + Matmul (~31 ns),
serialized on the in-order PE queue: two 33-col group-Grams cost 1024
LS+MM pairs (~64 us, the measured end-to-end pacer), one fused 65-col
Gram costs 512 (~32 us) for the same streamed column count - the
cross-group blocks are computed but ignored. L = sum(mask) is a pure
function of the mask, computed on host in the same pass that
narrows/marshals it; host also does the O(B) scalar epilogue.

The host shards AND marshals: all three inputs are laid out per-core in
the exact chunk-major SBUF staging order [p, c*32 + r] (a pure
permutation; y/p stay f32), and the mask is narrowed to int8 (it holds
0/1; int32 is 4x wasted HBM traffic). Device-side consequences:
 - every DMA is a fully contiguous [128, N] load (2-8 KiB runs per
   partition) on the two HWDGE rings (sync + scalar sequencers, zero Q7
   descriptor cost), ~9.1 MiB per ring per core;
 - the masked transpose-muls feeding the PE have contiguous-run inputs,
   so VectorE runs them at full 1x rate and handles all four (a/b x two
   row-groups) under the DMA budget. The int8 mask feeds the muls
   directly (DVE converts on read; a widen pass measured as pure loss).
   GpSimd stays idle: it shares SBUF ports with VectorE (concurrent
   Pool+DVE elementwise measured ~2x slowdown on BOTH engines).
Work is pipelined in 8 T-slice units so the PE starts early and the
drain is short.

HBM traffic per core: 18.25 MiB (y/p f32 16 MiB + mask i8 2 MiB + out).
"""

import numpy as np

import concourse.bass as bass
import concourse.tile as tile
from concourse import mybir
from concourse.bass_utils import run_bass_kernel_spmd

# ---------------------------------------------------------------- constants
B, T = 256, 65536
NCORES = 8
R = B // NCORES            # rows per core = 32
R2 = 16                    # rows per Gram column-group
NUNIT = 8                  # pipeline T-slice units
TU = T // NUNIT            # 8192 t per unit
CH = TU // 128             # chunk positions per row per unit = 64
GCOLS = 2 * R + 1          # 65 Gram columns: [a_g0, b_g0, ones, a_g1, b_g1]
KA = (0, 2 * R2 + 1)       # a-column base per group
KB = (R2, 3 * R2 + 1)      # b-column base per group
KONE = 2 * R2              # ones column

FP = mybir.dt.bfloat16     # Gram operand precision (PE-native, 1 cyc/col)


def split_multi_waits(nc: bass.Bass) -> int:
    """This container's walrus build accepts at most ONE sync-wait per
    instruction, but Tile's sem assignment attaches all required waits to
    the consuming instruction. Hoist the excess onto same-engine NoOps
    inserted immediately before it (sequencers execute in order, so the
    waits are still satisfied before the instruction issues)."""
    n_split = 0
    for f in nc.m.functions:
        for bb in f.blocks:
            insts = bb.instructions
            out = []
            for inst in insts:
                si = inst.sync_info
                if si is not None and si.on_wait and len(si.on_wait) > 1:
                    waits = list(si.on_wait)
                    for w in waits[:-1]:
                        nop = mybir.InstNoOp(
                            name=f"I-wsplit-{nc.next_id()}", ins=[], outs=[]
                        )
                        nop.engine = inst.engine
                        nop.sync_info = mybir.SyncInfo(on_wait=[w], on_update=[])
                        out.append(nop)
                        n_split += 1
                    inst.sync_info = mybir.SyncInfo(
                        on_wait=[waits[-1]], on_update=list(si.on_update or [])
                    )
                out.append(inst)
            bb.instructions = out
    return n_split


def build_nc() -> bass.Bass:
    nc = bass.Bass()
    # host-marshaled inputs, staged chunk-major per unit:
    # x[u*128 + p, c*R + r] = orig[r, u*TU + p*CH + c]
    yt = nc.dram_tensor("ypk", [NUNIT * 128, CH * R], mybir.dt.float32,
                        kind="ExternalInput")
    yp = nc.dram_tensor("ppk", [NUNIT * 128, CH * R], mybir.dt.float32,
                        kind="ExternalInput")
    mkp = nc.dram_tensor("maskp", [NUNIT * 128, CH * R], mybir.dt.int8,
                         kind="ExternalInput")
    gram = nc.dram_tensor("gram", [GCOLS, GCOLS], mybir.dt.float32,
                          kind="ExternalOutput")

    with tile.TileContext(nc) as tc:
        with (
            tc.tile_pool(name="stage", bufs=6) as stage,
            tc.tile_pool(name="mpool", bufs=6) as mpool,
            tc.tile_pool(name="gpool", bufs=2) as gpool,
            tc.tile_pool(name="psum", bufs=1, space="PSUM") as psum,
            tc.tile_pool(name="outp", bufs=1) as outp,
        ):
            ps = psum.tile([GCOLS, GCOLS], mybir.dt.float32)

            for u in range(NUNIT):
                rows = slice(u * 128, (u + 1) * 128)
                m8 = mpool.tile([128, CH * R], mybir.dt.int8)
                ty = stage.tile([128, CH * R], mybir.dt.float32)
                tp = stage.tile([128, CH * R], mybir.dt.float32)
                # contiguous loads; rings balanced to ~9.1 MiB each:
                # y + odd-unit masks on sync, p + even-unit masks on scalar
                mring = nc.scalar if u % 2 == 0 else nc.sync
                mring.dma_start(out=m8[:, :], in_=mkp[rows, :])
                nc.sync.dma_start(out=ty[:, :], in_=yt[rows, :])
                nc.scalar.dma_start(out=tp[:, :], in_=yp[rows, :])

                # G is chunk-major: G[p, ci*GCOLS + k] so each matmul
                # chunk's operand is contiguous
                gt = gpool.tile([128, CH * GCOLS], FP)
                gv = gt[:, :].rearrange("p (c k) -> p c k", k=GCOLS)
                nc.vector.memset(gv[:, :, KONE : KONE + 1], 1.0)
                # [p][c][r] views; inner 16-run contiguous per group slice
                cm = lambda t_: t_[:, :].rearrange("p (c r) -> p c r", r=R)
                tyv, tpv, m8v = cm(ty), cm(tp), cm(m8)
                for g in range(2):
                    rs = slice(g * R2, (g + 1) * R2)
                    nc.vector.tensor_mul(
                        out=gv[:, :, KA[g] : KA[g] + R2],
                        in0=tyv[:, :, rs],
                        in1=m8v[:, :, rs],
                    )
                    nc.vector.tensor_mul(
                        out=gv[:, :, KB[g] : KB[g] + R2],
                        in0=tpv[:, :, rs],
                        in1=m8v[:, :, rs],
                    )

                for ci in range(CH):
                    w = gt[:, ci * GCOLS : (ci + 1) * GCOLS]
                    nc.tensor.matmul(
                        ps[:, :],
                        lhsT=w,
                        rhs=w,
                        start=(u == 0 and ci == 0),
                        stop=(u == NUNIT - 1 and ci == CH - 1),
                    )

            og = outp.tile([GCOLS, GCOLS], mybir.dt.float32)
            nc.vector.tensor_copy(out=og[:, :], in_=ps[:, :])
            nc.sync.dma_start(out=gram[:, :], in_=og[:, :])
    split_multi_waits(nc)
    return nc


_NC_CACHE = None


def _get_nc():
    global _NC_CACHE
    if _NC_CACHE is None:
        _NC_CACHE = build_nc()
    return _NC_CACHE


def _pack(x: np.ndarray, dtype) -> np.ndarray:
    """[B, T] -> [NCORES, NUNIT*128, CH*R] in chunk-major staged layout:
    out[core, u*128 + p, c*R + r] = x[core*R + r, u*TU + p*CH + c]"""
    x = x.astype(dtype, copy=False)
    x = x.reshape(NCORES, R, NUNIT, 128, CH)
    x = x.transpose(0, 2, 3, 4, 1)  # core, u, p, c, r
    return np.ascontiguousarray(x.reshape(NCORES, NUNIT * 128, CH * R))


def _in_maps(y_true, y_pred, mask):
    return [
        {"ypk": yk, "ppk": pk, "maskp": mk}
        for yk, pk, mk in zip(
            _pack(np.asarray(y_true), np.float32),
            _pack(np.asarray(y_pred), np.float32),
            _pack(np.asarray(mask), np.int8),
        )
    ]


def _ccc_from_outputs(results, ell_all) -> np.ndarray:
    idx = np.arange(R2)
    total = 0.0
    for core, res in enumerate(results):
        gg = res["gram"].astype(np.float64)
        for g in range(2):
            ka, kb = KA[g], KB[g]
            s2t = gg[ka + idx, ka + idx]
            stp = gg[ka + idx, kb + idx]
            s1t = gg[ka + idx, KONE]
            s2p = gg[kb + idx, kb + idx]
            s1p = gg[kb + idx, KONE]
            ell = ell_all[core * R + g * R2 : core * R + (g + 1) * R2]
            mean_t = s1t / ell
            mean_p = s1p / ell
            denom = ell - 1.0
            var_t = (s2t - s1t * s1t / ell) / denom
            var_p = (s2p - s1p * s1p / ell) / denom
            cov = (stp - s1t * s1p / ell) / denom
            ccc = 2.0 * cov / (var_t + var_p + (mean_t - mean_p) * 2.0)
            total += ccc.sum()
    return np.float32(total / B)


def kernel(y_true, y_pred, mask) -> np.ndarray:
    mask = np.asarray(mask)
    # per-row valid length: a pure function of the mask, folded into the
    # same host pass that narrows/marshals it
    ell = mask.sum(axis=1, dtype=np.int64).astype(np.float64)
    nc = _get_nc()
    res = run_bass_kernel_spmd(
        nc, _in_maps(y_true, y_pred, mask), core_ids=list(range(NCORES))
    )
    return _ccc_from_outputs(res.results, ell)
